# revision 21
# baseline (speedup 1.0000x reference)
"""Fused LayerNorm->MHA(multi-query)->LayerNorm kernel for TRN2, 8 cores SPMD.

Problem shapes (hardcoded):
  x:        [4, 2048, 512] f32
  attn_bias:[8, 2048, 2048] f32   (shared across batch)
  w_q:      [512, 512], w_kv: [512, 128], w_out: [512, 512]
  g_in, g_out: [512]
  out:      [4, 2048, 512] f32

Sharding: 8 cores = (batch b in 0..3) x (query-half ih in 0..1).
Each core computes the full pipeline for one batch and 1024 query rows:
  - LayerNorm(x[b]) for all 2048 rows (k/v need every position)
  - q projection only for its 1024 rows
  - S^T[j, i] = k[j] . q[i] per head, + bias^T, exp (softmax without max
    subtraction: |sim| <= ~7 so exp is safe in fp32)
  - denominator via an appended ones-column in v (row 64 of the A@V output)
  - out = attn @ v, normalized, projected by w_out, LayerNorm, * g_out

On-chip layouts: "T" tensors keep the contraction dim on partitions.
attn_bias is pre-transposed on host to [h, j, i] so the per-core DMA
slices are contiguous 1MiB chunks.

Host-side folding: g_in and the q scale (dim_head^-0.5) are folded into
w_q / w_kv; w_kv columns are reordered to [v, k] so the v row-tiles can be
built by PE transpose from partition base 0.
"""

import sys

sys.path.insert(0, "/opt/trn_rl_repo")

import numpy as np
from contextlib import ExitStack

import concourse.bass as bass
import concourse.tile as tile
from concourse import bacc
from concourse import mybir
from concourse.masks import make_identity

B, N, DIM = 4, 2048, 512
HEADS, DH = 8, 64
INNER = HEADS * DH  # 512
EPS = 1e-5
SCALE = DH ** -0.5
NCORES = 8
IH = N // 2  # 1024 query rows per core
P = 128

NT = N // P     # 16 row tiles of x / j tiles
DT = DIM // P   # 4 d tiles
CT = INNER // P  # 4 c tiles (head pairs)
ICH = IH // 512  # 2 i chunks of 512
JT = N // P     # 16 j tiles
JPAIR = 4       # j tiles per bias DMA chunk (1 MiB fp16 chunks)

F32 = mybir.dt.float32
F32R = mybir.dt.float32r
F16 = mybir.dt.float16
BF16 = mybir.dt.bfloat16

# dtype of the attn_bias stream (the dominant HBM traffic).
# float16 halves it; quantization error ~2.5e-3 absolute on sim, ~6e-4
# relative on the final output.
BIAS_DT = mybir.dt.float16


def _mm_dt(ap):
    """fp32r tensors are used directly; kept for call-site clarity."""
    return ap


def build_bass():
    nc = bacc.Bacc("TRN2")
    x_d = nc.dram_tensor("x", [N, DIM], F32, kind="ExternalInput")
    bias_d = nc.dram_tensor("biasT", [HEADS, N, IH], BIAS_DT, kind="ExternalInput")
    wq_d = nc.dram_tensor("wq", [DIM, INNER], F16, kind="ExternalInput")
    wkv_d = nc.dram_tensor("wkv", [DIM, 2 * DH], F16, kind="ExternalInput")
    wout_d = nc.dram_tensor("wout", [INNER, DIM], F16, kind="ExternalInput")
    gout_d = nc.dram_tensor("gout", [DIM], F32, kind="ExternalInput")
    out_d = nc.dram_tensor("out", [IH, DIM], F32, kind="ExternalOutput")

    with tile.TileContext(nc) as tc:
        _body(tc, x_d, bias_d, wq_d, wkv_d, wout_d, gout_d, out_d)
    nc.compile()
    return nc


def _body(tc, x_d, bias_d, wq_d, wkv_d, wout_d, gout_d, out_d):
    nc = tc.nc
    ctx = ExitStack()
    with ctx:
        # long-lived tensors
        persist = ctx.enter_context(tc.tile_pool(name="persist", bufs=1))

        # issue the x loads before anything else so LayerNorm can start
        # immediately (bias prefetch otherwise hogs the DMA queues)
        xload = ctx.enter_context(tc.tile_pool(name="xload", bufs=1))
        x_tiles = []
        for nt in range(NT):
            x_t = xload.tile([P, DIM], F32, name=f"x{nt}")
            nc.sync.dma_start(out=x_t, in_=x_d[nt * P:(nt + 1) * P, :])
            x_tiles.append(x_t)

        identity = persist.tile([P, P], F16, name="identity")
        make_identity(nc, identity)
        eps_t = persist.tile([P, 1], F32, name="eps")
        nc.vector.memset(eps_t, EPS)
        gout_bc = persist.tile([P, DIM], F32, name="gout_bc")
        gout_ap = gout_d[:]
        nc.sync.dma_start(
            out=gout_bc,
            in_=bass.AP(tensor=gout_ap.tensor, offset=gout_ap.offset,
                        ap=[[0, P], [1, DIM]]),
        )

        # weights
        wq_sb = [persist.tile([P, INNER], F16, name=f"wq{d}") for d in range(DT)]
        wkv_sb = [persist.tile([P, 2 * DH], F16, name=f"wkv{d}") for d in range(DT)]
        wout_sb = [persist.tile([DH, DIM], F16, name=f"wout{h}")
                   for h in range(HEADS)]
        for d in range(DT):
            nc.sync.dma_start(out=wq_sb[d], in_=wq_d[d * P:(d + 1) * P, :])
            nc.sync.dma_start(out=wkv_sb[d], in_=wkv_d[d * P:(d + 1) * P, :])
        for h in range(HEADS):
            nc.sync.dma_start(out=wout_sb[h], in_=wout_d[h * DH:(h + 1) * DH, :])

        # xn^T (d on partitions), k^T duplicated to both partition halves,
        # v-with-ones row tiles, q^T (c on partitions), attnout^T per head
        xnT = [persist.tile([P, N], F16, name=f"xnT{d}") for d in range(DT)]
        kT2 = persist.tile([P, N], F16, name="kT2")
        vp = [persist.tile([P, DH + 1], F16, name=f"vp{j}") for j in range(JT)]
        qT = [persist.tile([P, IH], F16, name=f"qT{t}") for t in range(CT)]
        aoT = [persist.tile([DH, 2 * IH], F16, name=f"aoT{t}") for t in range(CT)]

        # ---- Phase 1: LayerNorm(x) and transpose into xnT ----
        with tc.tile_pool(name="ln", bufs=3) as ln, \
             tc.tile_pool(name="lnps", bufs=4, space="PSUM") as lnps:
            for nt in range(NT):
                x_t = x_tiles[nt]
                stats = ln.tile([P, 6], F32, name="stats")
                nc.vector.bn_stats(out=stats, in_=x_t)
                mv = ln.tile([P, 2], F32, name="mv")
                nc.vector.bn_aggr(out=mv, in_=stats)
                mean = mv[:, 0:1]
                rstd = ln.tile([P, 1], F32, name="rstd")
                nc.scalar.activation(
                    out=rstd, in_=mv[:, 1:2],
                    func=mybir.ActivationFunctionType.Sqrt,
                    bias=eps_t, scale=1.0)
                nc.vector.reciprocal(out=rstd, in_=rstd)
                # xn = x * rstd + (-mean * rstd), applied on the (otherwise
                # idle) scalar engine with per-partition scale/bias
                negmr = ln.tile([P, 1], F32, name="negmr")
                nc.vector.tensor_scalar(
                    out=negmr, in0=mv[:, 0:1], scalar1=rstd, scalar2=-1.0,
                    op0=mybir.AluOpType.mult, op1=mybir.AluOpType.mult)
                xn_t = ln.tile([P, DIM], F16, name="xn_t")
                nc.scalar.activation(
                    out=xn_t, in_=x_t,
                    func=mybir.ActivationFunctionType.Identity,
                    bias=negmr, scale=rstd)
                for d in range(DT):
                    ps = lnps.tile([P, P], F16, name="tps")
                    nc.tensor.transpose(ps, xn_t[:, d * P:(d + 1) * P], identity)
                    nc.vector.tensor_copy(
                        out=xnT[d][:, nt * P:(nt + 1) * P], in_=ps)

        # ---- Phase 2: projections (kv, q), v row tiles, kT duplication ----
        with tc.tile_pool(name="proj", bufs=3) as proj, \
             tc.tile_pool(name="projps", bufs=2, space="PSUM") as projps:
            # kvT[c, n]: c = [v(64), k(64)] on partitions (host-swapped)
            kvT = proj.tile([P, N], F16, name="kvT")
            for nch in range(N // 512):
                ps = projps.tile([P, 512], F32, name="kvps")
                for d in range(DT):
                    nc.tensor.matmul(
                        ps, _mm_dt(wkv_sb[d]),
                        _mm_dt(xnT[d][:, nch * 512:(nch + 1) * 512]),
                        start=(d == 0), stop=(d == DT - 1))
                nc.vector.tensor_copy(out=kvT[:, nch * 512:(nch + 1) * 512], in_=ps)
            # k rows live at partitions 64:128 of kvT; duplicate into both
            # halves of kT2 for 2-head row packing.  The partition-shifting
            # copy (64:128 -> 0:64) must go through DMA.
            nc.vector.tensor_copy(out=kT2[DH:2 * DH, :], in_=kvT[DH:2 * DH, :])
            nc.sync.dma_start(out=kT2[0:DH, :], in_=kvT[DH:2 * DH, :])
            # v row tiles with appended ones column
            for j in range(JT):
                ps = projps.tile([P, DH], F16, name="vps")
                nc.tensor.transpose(
                    ps, kvT[0:DH, j * P:(j + 1) * P], identity[0:DH, 0:DH])
                nc.vector.tensor_copy(out=vp[j][:, 0:DH], in_=ps)
                nc.vector.memset(vp[j][:, DH:DH + 1], 1.0)
            # qT[c, i] for our 1024 query rows (x rows are host-permuted so
            # the local query half is always rows 0:1024)
            for t in range(CT):
                for ic in range(ICH):
                    ps = projps.tile([P, 512], F32, name="qps")
                    for d in range(DT):
                        nc.tensor.matmul(
                            ps, _mm_dt(wq_sb[d][:, t * P:(t + 1) * P]),
                            _mm_dt(xnT[d][:, ic * 512:(ic + 1) * 512]),
                            start=(d == 0), stop=(d == DT - 1))
                    nc.vector.tensor_copy(
                        out=qT[t][:, ic * 512:(ic + 1) * 512], in_=ps)

        # ---- Phase 3: attention, head pairs ----
        # S stays fp32 in PSUM ([128, 1024] = 2 banks per (head, j-tile)).
        # The additive bias is applied as a multiplicative exp(bias)
        # (precomputed on host, streamed fp16) AFTER the exp, on DVE in
        # 2x 16-bit mode.  exp writes fp16; A@V runs in fp16.
        with tc.tile_pool(name="bias", bufs=4) as biasp, \
             tc.tile_pool(name="attn", bufs=6) as attnp, \
             tc.tile_pool(name="den", bufs=4) as denp, \
             tc.tile_pool(name="dden", bufs=4, space="DRAM") as ddenp, \
             tc.tile_pool(name="qkps", bufs=2, space="PSUM") as qkps, \
             tc.tile_pool(name="avps", bufs=1, space="PSUM") as avps:
            for hp in range(HEADS // 2):
                av = [[avps.tile([DH + 1, 512], F32, name=f"av{hh}_{ic}")
                       for ic in range(ICH)] for hh in range(2)]
                pend = None
                for jp in range(JT // JPAIR):
                    eb_t = [None, None]
                    for hh in range(2):
                        h = 2 * hp + hh
                        eb_t[hh] = biasp.tile([P, JPAIR, IH], BIAS_DT,
                                              name=f"bias{hh}")
                        nc.sync.dma_start(
                            out=eb_t[hh],
                            in_=bias_d[h, jp * JPAIR * P:(jp + 1) * JPAIR * P, :]
                            .rearrange("(t p) i -> p t i", p=P))
                    for jj in range(JPAIR):
                        j = jp * JPAIR + jj
                        # 1) this unit's QK matmuls (PE)
                        s_big = [None, None]
                        for hh in range(2):
                            s_big[hh] = qkps.tile([P, ICH, 512], F32,
                                                  name="s_big")
                            for ic in range(ICH):
                                nc.tensor.matmul(
                                    s_big[hh][:, ic, :],
                                    _mm_dt(kT2[hh * DH:(hh + 1) * DH,
                                               j * P:(j + 1) * P]),
                                    _mm_dt(qT[hp][hh * DH:(hh + 1) * DH,
                                                  ic * 512:(ic + 1) * 512]),
                                    start=True, stop=True,
                                    tile_position=(hh * DH, 0))
                        # 2) previous unit's A@V (PE) — its e_t is ready, so
                        # the in-order PE never stalls on DVE here
                        if pend is not None:
                            pj, pe_pair = pend
                            for hh in range(2):
                                for ic in range(ICH):
                                    nc.tensor.matmul(
                                        av[hh][ic], vp[pj],
                                        pe_pair[hh][:, ic, :],
                                        start=(pj == 0), stop=(pj == JT - 1))
                            pend = None
                        # 3) this unit's exp (ACT) + exp-bias multiply (DVE)
                        e_pair = [None, None]
                        for hh in range(2):
                            e_t = attnp.tile([P, ICH, 512], F16, name="e_t")
                            nc.scalar.activation(
                                out=e_t, in_=s_big[hh],
                                func=mybir.ActivationFunctionType.Exp)
                            eb_slice = eb_t[hh][:, jj, :].rearrange(
                                "p (c u) -> p c u", c=ICH)
                            nc.vector.tensor_tensor(
                                e_t, e_t, eb_slice, mybir.AluOpType.mult)
                            e_pair[hh] = e_t
                        pend = (j, e_pair)
                # drain the last pipelined unit
                if pend is not None:
                    pj, pe_pair = pend
                    for hh in range(2):
                        for ic in range(ICH):
                            nc.tensor.matmul(
                                av[hh][ic], vp[pj], pe_pair[hh][:, ic, :],
                                start=(pj == 0), stop=(pj == JT - 1))
                    pend = None
                # normalize by the softmax denominator (row DH of av).
                # Copy out of PSUM quickly to release the banks, then do the
                # reciprocal in a [128, 4] layout (a [1, 512] DVE op uses a
                # single lane and takes ~3.3us) via small DRAM round-trips.
                for hh in range(2):
                    for ic in range(ICH):
                        rd = denp.tile([1, 512], F32, name="rd")
                        nc.vector.tensor_copy(out=rd, in_=av[hh][ic][DH:DH + 1, :])
                        ao_un = denp.tile([DH, 512], F32, name="ao_un")
                        nc.vector.tensor_copy(out=ao_un, in_=av[hh][ic][0:DH, :])
                        dden = ddenp.tile([1, 512], F32, name="dden")
                        nc.sync.dma_start(out=dden, in_=rd)
                        dd_ap = dden[:] if not isinstance(dden, bass.AP) else dden
                        # scatter to [128, 4], reciprocal in parallel, gather
                        denT = denp.tile([P, 4], F32, name="denT")
                        nc.sync.dma_start(
                            out=denT,
                            in_=bass.AP(tensor=dd_ap.tensor, offset=dd_ap.offset,
                                        ap=[[4, P], [1, 4]]))
                        nc.vector.reciprocal(out=denT, in_=denT)
                        dden2 = ddenp.tile([1, 512], F32, name="dden2")
                        dd2_ap = dden2[:] if not isinstance(dden2, bass.AP) else dden2
                        nc.sync.dma_start(
                            out=bass.AP(tensor=dd2_ap.tensor, offset=dd2_ap.offset,
                                        ap=[[4, P], [1, 4]]),
                            in_=denT)
                        bc = denp.tile([DH, 512], F32, name="bc")
                        nc.sync.dma_start(
                            out=bc,
                            in_=bass.AP(tensor=dd2_ap.tensor, offset=dd2_ap.offset,
                                        ap=[[0, DH], [1, 512]]))
                        nc.vector.tensor_tensor(
                            aoT[hp][:, hh * IH + ic * 512:
                                    hh * IH + (ic + 1) * 512],
                            ao_un, bc, mybir.AluOpType.mult)

        # ---- Phase 4: output projection + LayerNorm * g_out ----
        with tc.tile_pool(name="fin", bufs=3) as fin, \
             tc.tile_pool(name="finps", bufs=4, space="PSUM") as finps:
            for it in range(IH // P):
                o_ps = finps.tile([P, DIM], F32, name="o_ps")
                for h in range(HEADS):
                    t, hh = divmod(h, 2)
                    nc.tensor.matmul(
                        o_ps,
                        _mm_dt(aoT[t][:, hh * IH + it * P:
                                      hh * IH + (it + 1) * P]),
                        _mm_dt(wout_sb[h]),
                        start=(h == 0), stop=(h == HEADS - 1))
                stats = fin.tile([P, 6], F32, name="stats")
                nc.vector.bn_stats(out=stats, in_=o_ps)
                mv = fin.tile([P, 2], F32, name="mv")
                nc.vector.bn_aggr(out=mv, in_=stats)
                rstd = fin.tile([P, 1], F32, name="rstd")
                nc.scalar.activation(
                    out=rstd, in_=mv[:, 1:2],
                    func=mybir.ActivationFunctionType.Sqrt,
                    bias=eps_t, scale=1.0)
                nc.vector.reciprocal(out=rstd, in_=rstd)
                negmr = fin.tile([P, 1], F32, name="negmr")
                nc.vector.tensor_scalar(
                    out=negmr, in0=mv[:, 0:1], scalar1=rstd, scalar2=-1.0,
                    op0=mybir.AluOpType.mult, op1=mybir.AluOpType.mult)
                o_sb = fin.tile([P, DIM], F32, name="o_sb")
                nc.scalar.activation(
                    out=o_sb, in_=o_ps,
                    func=mybir.ActivationFunctionType.Identity,
                    bias=negmr, scale=rstd)
                nc.vector.tensor_tensor(o_sb, o_sb, gout_bc,
                                        mybir.AluOpType.mult)
                nc.sync.dma_start(out=out_d[it * P:(it + 1) * P, :], in_=o_sb)


_NC_CACHE = None


def _get_nc():
    global _NC_CACHE
    if _NC_CACHE is None:
        _NC_CACHE = build_bass()
    return _NC_CACHE


def make_in_maps(x, attn_bias, w_q, w_kv, w_out, g_in, g_out):
    x = np.asarray(x, np.float32)
    attn_bias = np.asarray(attn_bias, np.float32)
    g_in = np.asarray(g_in, np.float32)
    wq_eff = np.ascontiguousarray(
        ((g_in[:, None] * np.asarray(w_q, np.float32)) * SCALE).astype(np.float16))
    wkv = g_in[:, None] * np.asarray(w_kv, np.float32)
    # reorder kv projection columns to [v, k]
    wkv_eff = np.ascontiguousarray(
        np.concatenate([wkv[:, DH:], wkv[:, :DH]], axis=1).astype(np.float16))
    w_out = np.ascontiguousarray(
        np.asarray(w_out, np.float32).astype(np.float16))
    g_out = np.ascontiguousarray(np.asarray(g_out, np.float32))
    np_bias_dt = mybir.dt.np(BIAS_DT)
    biasT = np.ascontiguousarray(
        np.exp(np.transpose(attn_bias, (0, 2, 1))).astype(np_bias_dt))  # exp(bias) [h, j, i]
    in_maps = []
    for c in range(NCORES):
        b, ih = divmod(c, 2)
        lo, hi = ih * IH, (ih + 1) * IH
        # local query rows first; k/v row order is irrelevant to the math
        # as long as the bias j-rows are permuted identically
        xp = np.concatenate([x[b, lo:hi], x[b, :lo], x[b, hi:]], axis=0)
        bj = np.concatenate(
            [biasT[:, lo:hi, lo:hi], biasT[:, :lo, lo:hi], biasT[:, hi:, lo:hi]],
            axis=1)
        in_maps.append({
            "x": np.ascontiguousarray(xp),
            "biasT": np.ascontiguousarray(bj),
            "wq": wq_eff, "wkv": wkv_eff, "wout": w_out, "gout": g_out,
        })
    return in_maps


def assemble(results):
    out = np.empty((B, N, DIM), np.float32)
    for c in range(NCORES):
        b, ih = divmod(c, 2)
        out[b, ih * IH:(ih + 1) * IH, :] = results[c]["out"]
    return out


def kernel(x, attn_bias, w_q, w_kv, w_out, g_in, g_out):
    from concourse.bass_utils import run_bass_kernel_spmd

    in_maps = make_in_maps(x, attn_bias, w_q, w_kv, w_out, g_in, g_out)
    nc = _get_nc()
    res = run_bass_kernel_spmd(nc, in_maps, list(range(NCORES))).results
    return assemble(res)
